# revision 41
# baseline (speedup 1.0000x reference)
"""Trainium2 Bass kernel for nn_AttentionalGNN (8-core SPMD, sequence-sharded).

Strategy:
  - Shard the N=1024 node axis across 8 cores (128 queries each). All of
    attention/merge/MLP/LayerNorm is local per position; only k/v need the full
    sequence, shared via fused AllGathers (2 per layer, software-pipelined so
    the second AG overlaps the first half's attention compute).
  - Heads are made contiguous by permuting projection output channels host-side
    (orig channel d = 4a+b -> row b*64+a), with the merge weight columns
    permuted to match.
  - Scores are computed per-head with K=64 row-tiled matmuls (two heads run
    concurrently in the PE array's upper/lower row groups); exp is batched as
    one activation over a 2-bank PSUM score buffer; the softmax denominator is
    a bf16 tree-reduction on DVE plus one ones-matmul partition reduce.
  - The final output only uses the layer-5 scores of prop(0,5,d1,d0), reduced
    over heads and queries: out[m] = (1/32) * (Wq5 @ mean_n d1)^T (Wk5 @ d0)[:,m].
    So layer 5 collapses to two projections + a tiny cross-core reduce.
"""
import numpy as np

import concourse.bass as bass
import concourse.bacc as bacc
import concourse.mybir as mybir
import concourse.tile as tile
from concourse.bass_utils import run_bass_kernel_spmd

D, N, H, DH = 256, 1024, 4, 64
NC = 8
CH = N // NC  # 128 positions per core
F32 = mybir.dt.float32
BF16 = mybir.dt.bfloat16
AF = mybir.ActivationFunctionType

PROPS_SELF = [(0, 0, 0), (0, 1, 1), (1, 2, 2), (2, 3, 3)]
PROPS_CROSS = [(0, 0, 1), (0, 1, 0), (1, 2, 1), (1, 1, 2), (2, 0, 3), (2, 3, 0)]
NAMES5 = ['self', 'cross', 'self', 'cross', 'self']
PERM = np.array([4 * (r % 64) + r // 64 for r in range(256)])

_cache = {}


def _props(i):
    return PROPS_CROSS if NAMES5[i] == 'cross' else PROPS_SELF


def build_kernel(trace_scopes=False, n_layers=5, stages="ABC", blvl=9, reps=1,
                 zb=True, agsplit=3, expbig=False, rowsc=False, sc1b=True):
    if sc1b:
        expbig = False
    nc = bacc.Bacc("TRN2", target_bir_lowering=False, debug=False, num_devices=NC)

    # ---- I/O ----
    xc = nc.dram_tensor("xc", [4, 2, 128, CH], F32, kind="ExternalInput")
    wqkvT = nc.dram_tensor("wqkvT", [5, 3, 3, 256, 256], BF16, kind="ExternalInput")
    mergeT = nc.dram_tensor("mergeT", [5, 3, 256, 256], BF16, kind="ExternalInput")
    w1T = nc.dram_tensor("w1T", [5, 3, 512, 512], BF16, kind="ExternalInput")
    w2T = nc.dram_tensor("w2T", [5, 3, 512, 256], BF16, kind="ExternalInput")
    pbq = nc.dram_tensor("pbq", [5, 3, 256], F32, kind="ExternalInput")
    pbk = nc.dram_tensor("pbk", [5, 3, 256], F32, kind="ExternalInput")
    pbv = nc.dram_tensor("pbv", [5, 3, 256], F32, kind="ExternalInput")
    mbb = nc.dram_tensor("mbb", [5, 3, 256], F32, kind="ExternalInput")
    b1b = nc.dram_tensor("b1b", [5, 3, 512], F32, kind="ExternalInput")
    b2b = nc.dram_tensor("b2b", [5, 3, 256], F32, kind="ExternalInput")
    lng = nc.dram_tensor("lng", [5, 4, 256], F32, kind="ExternalInput")
    lnb = nc.dram_tensor("lnb", [5, 4, 256], F32, kind="ExternalInput")
    w5T = nc.dram_tensor("w5T", [2, 256, 256], F32, kind="ExternalInput")  # [qT,kT]
    pb5 = nc.dram_tensor("pb5", [2, 256], F32, kind="ExternalInput")
    out_d = nc.dram_tensor("out", [1, CH], F32, kind="ExternalOutput")

    # per-(layer, group) collective buffers (compile-time static)
    # chunk layout per prop (65536 elems): [0:16384] k c=0 rows(p)=dims0..127,
    # [16384:32768] k c=1, [32768:65536] vT [128 keys, 256 dims]
    groups = []  # groups[i] = list of (props_slice, agin, agout)
    for i in range(5):
        props = _props(i)
        if agsplit == 2:
            half = (len(props) + 1) // 2
        elif agsplit >= 3:
            half = 2
        else:
            half = len(props)
        gs = []
        for g, lo in enumerate(range(0, len(props), half)):
            grp = props[lo:lo + half]
            agin = nc.dram_tensor(f"agin{i}_{g}", [len(grp), 65536], BF16)
            agout = nc.dram_tensor(f"agout{i}_{g}", [NC, len(grp), 65536], BF16,
                                   addr_space="Shared")
            gs.append((lo, grp, agin, agout))
        groups.append(gs)
    ag2in = nc.dram_tensor("ag2in", [2, 128, 1], F32)
    ag2out = nc.dram_tensor("ag2out", [NC, 2, 128, 1], F32, addr_space="Shared")

    # persistent SBUF state
    dst = nc.alloc_sbuf_tensor("dstate", [128, 4, 2, CH], F32)   # d[t] ctile c
    dlt = nc.alloc_sbuf_tensor("delta", [128, 4, 2, CH], F32)
    dstb = nc.alloc_sbuf_tensor("dstateb", [128, 4, 2, CH], BF16)

    rg = [list(range(NC))]

    from contextlib import ExitStack
    with ExitStack() as es:
        tc = es.enter_context(tile.TileContext(nc))
        cpool = es.enter_context(tc.tile_pool(name="const", bufs=1))
        qkvp = es.enter_context(tc.tile_pool(name="qkv", bufs=4))
        mgp = es.enter_context(tc.tile_pool(name="mg", bufs=2))
        w1p = es.enter_context(tc.tile_pool(name="w1", bufs=2))
        w2p = es.enter_context(tc.tile_pool(name="w2", bufs=2))
        bp = es.enter_context(tc.tile_pool(name="bias", bufs=3))
        khp = es.enter_context(tc.tile_pool(name="kh", bufs=3))
        vtp = es.enter_context(tc.tile_pool(name="vt", bufs=3))
        ap_ = es.enter_context(tc.tile_pool(name="act", bufs=4))
        ep = es.enter_context(tc.tile_pool(name="exps", bufs=4))
        zpl = es.enter_context(tc.tile_pool(name="zs", bufs=2))
        sp_ = es.enter_context(tc.tile_pool(name="small", bufs=4))
        ps = es.enter_context(tc.tile_pool(name="ps", bufs=2, space="PSUM"))
        scp = es.enter_context(tc.tile_pool(name="sc", bufs=2, space="PSUM"))
        avp = es.enter_context(tc.tile_pool(name="av", bufs=2, space="PSUM"))
        if True:
            ones_c = cpool.tile([128, 1], F32, name="tl", tag="ones_c")
            nc.gpsimd.memset(ones_c[:], 1.0)
            ones_r = cpool.tile([1, 128], F32, name="tl", tag="ones_r")
            nc.gpsimd.memset(ones_r[:], 1.0)
            eps_c = cpool.tile([1, 1], F32, name="tl", tag="eps_c")
            nc.gpsimd.memset(eps_c[:], 1e-5)
            ones64 = cpool.tile([128, 64], F32, name="tl", tag="ones64")
            nc.gpsimd.memset(ones64[:], 1.0)
            ones64b = cpool.tile([128, 64], BF16, name="tl", tag="ones64b")
            nc.gpsimd.memset(ones64b[:], 1.0)
            qpad = None
            if not rowsc:
                qpad = [cpool.tile([128, 256], BF16, name="tl", tag=f"qpad{c}") for c in range(2)]
                for c in range(2):
                    nc.gpsimd.memset(qpad[c][:], 0.0)

            # load descriptor chunks into d-state
            for t in range(4):
                for c in range(2):
                    nc.sync.dma_start(dst[:, t, c, :], xc[t, c])
                nc.vector.tensor_copy(dstb[:, t, :, :], dst[:, t, :, :])

            def load_qkv(i, br):
                t_ = qkvp.tile([128, 1536], BF16, name="tl", tag="qkv")
                nc.sync.dma_start(t_[:].rearrange("p (a f) -> p a f", a=6),
                                  wqkvT[i, br].rearrange("j (k p) f -> p (j k) f", p=128))
                return t_

            def load_wide(pool, src_ap, k, f, tag):
                t_ = pool.tile([128, k * f], BF16, name="tl", tag=tag)
                nc.sync.dma_start(t_[:].rearrange("p (k f) -> p k f", k=k),
                                  src_ap.rearrange("(k p) f -> p k f", p=128))
                return t_

            def load_bias(src_ap, n, tag):
                t_ = bp.tile([128, n // 128], F32, name="tl", tag=tag)
                nc.sync.dma_start(t_[:], src_ap.rearrange("(a p) -> p a", p=128))
                return t_

            for i in [li for _r in range(reps) for li in range(n_layers)]:
                props = _props(i)
                qkv_tiles = {}

                # ---- stage A: per group produce local k/v chunks, then AllGather
                for lo, grp, agin, agout in (groups[i] if "A" in stages else []):
                    cur_br = None
                    bk_t = bv_b = None
                    for gi, (br, xi, si) in enumerate(grp):
                        if br not in qkv_tiles:
                            qkv_tiles[br] = load_qkv(i, br)
                        qkv = qkv_tiles[br]
                        if br != cur_br and not zb:
                            cur_br = br
                            bk_t = load_bias(pbk[i, br], 256, "bk")
                            bvr = sp_.tile([1, 256], F32, name="tl", tag="bvr")
                            nc.sync.dma_start(bvr[:], pbv[i, br][None, :])
                            bv_ps = ps.tile([128, 256], F32, name="tl", tag="ps")
                            nc.tensor.matmul(bv_ps[:], ones_r[:], bvr[:], start=True, stop=True)
                            bv_b = sp_.tile([128, 256], F32, name="tl", tag="bvb")
                            nc.vector.tensor_copy(bv_b[:], bv_ps[:])
                        # k chunk: [256, CH] as 2 ctiles
                        for c in range(2):
                            kc_ps = ps.tile([128, CH], F32, name="tl", tag="ps")
                            for cc in range(2):
                                nc.tensor.matmul(kc_ps[:],
                                                 qkv[:, 512 + cc * 256 + c * 128:512 + cc * 256 + (c + 1) * 128],
                                                 dstb[:, si, cc, :], start=(cc == 0), stop=(cc == 1))
                            kc_sb = ap_.tile([128, CH], BF16, name="tl", tag="kcsb")
                            if zb:
                                nc.vector.tensor_copy(kc_sb[:], kc_ps[:])
                            else:
                                nc.scalar.activation(kc_sb[:], kc_ps[:], AF.Identity, bias=bk_t[:, c:c + 1])
                            nc.sync.dma_start(
                                agin[gi, c * 16384:(c + 1) * 16384].rearrange("(p f) -> p f", p=128),
                                kc_sb[:])
                        # vT chunk: [CH, 256]
                        vt_ps = ps.tile([128, 256], F32, name="tl", tag="ps")
                        for cc in range(2):
                            nc.tensor.matmul(vt_ps[:], dstb[:, si, cc, :],
                                             qkv[:, 1024 + cc * 256:1024 + (cc + 1) * 256],
                                             start=(cc == 0), stop=(cc == 1))
                        vt_sb = ap_.tile([128, 256], BF16, name="tl", tag="vtsb")
                        if zb:
                            nc.vector.tensor_copy(vt_sb[:], vt_ps[:])
                        else:
                            nc.vector.tensor_add(vt_sb[:], vt_ps[:], bv_b[:])
                        nc.sync.dma_start(
                            agin[gi, 32768:65536].rearrange("(p f) -> p f", p=128),
                            vt_sb[:])
                    nc.gpsimd.collective_compute(
                        "AllGather", mybir.AluOpType.bypass, replica_groups=rg,
                        ins=[agin.ap().opt()], outs=[agout.ap().opt()])

                # ---- stage B: per prop attention + merge + MLP on local queries
                first_delta = {t: True for t in range(4)}
                cur_br = None
                wq_t = mg_t = w1_t = w2_t = None
                bq_t = mb_t = b1_t = b2_t = None
                for lo, grp, agin, agout in (groups[i] if "B" in stages else []):
                    for gi, (br, xi, si) in enumerate(grp):
                        if br != cur_br:
                            cur_br = br
                            if br not in qkv_tiles:
                                qkv_tiles[br] = load_qkv(i, br)
                            qkv = qkv_tiles[br]
                            mg_t = load_wide(mgp, mergeT[i, br], 2, 256, "mg")
                            w1_t = load_wide(w1p, w1T[i, br], 4, 512, "w1")
                            w2_t = load_wide(w2p, w2T[i, br], 4, 256, "w2")
                            if not zb:
                                bq_t = load_bias(pbq[i, br], 256, "bq")
                                mb_t = load_bias(mbb[i, br], 256, "mb")
                                b1_t = load_bias(b1b[i, br], 512, "b1")
                                b2_t = load_bias(b2b[i, br], 256, "b2")

                        # gathered k: [128, c(2) x r(8) x 128] ; vT: [128, m(8) x 256]
                        kh_all = khp.tile([128, 2048], BF16, name="tl", tag="kh")
                        for c in range(2):
                            nc.sync.dma_start(
                                kh_all[:, c * 1024:(c + 1) * 1024].rearrange("p (r f) -> p r f", r=NC),
                                agout[:, gi, c * 16384:(c + 1) * 16384].rearrange("r (p f) -> p r f", p=128))
                        vt_all = vtp.tile([128, 2048], BF16, name="tl", tag="vt")
                        nc.sync.dma_start(
                            vt_all[:].rearrange("p (m f) -> p m f", m=NC),
                            agout[:, gi, 32768:65536].rearrange("m (p f) -> p m f", p=128))

                        # q: [256, CH] scaled by 1/8 (folded in weights host-side)
                        q_sb = [sp_.tile([128, CH], BF16, name="tl", tag=f"qsb{c}") for c in range(2)] \
                            if rowsc else None
                        for c in range(2):
                            q_ps = ps.tile([128, CH], F32, name="tl", tag="ps")
                            for cc in range(2):
                                nc.tensor.matmul(q_ps[:],
                                                 qkv[:, cc * 256 + c * 128:cc * 256 + (c + 1) * 128],
                                                 dstb[:, xi, cc, :], start=(cc == 0), stop=(cc == 1))
                            if rowsc:
                                if zb:
                                    nc.vector.tensor_copy(q_sb[c][:], q_ps[:])
                                else:
                                    nc.scalar.activation(q_sb[c][:], q_ps[:], AF.Identity,
                                                         bias=bq_t[:, c:c + 1])
                            else:
                                if zb:
                                    nc.vector.tensor_copy(qpad[c][0:64, 0:CH], q_ps[0:64, :])
                                    nc.vector.tensor_copy(qpad[c][64:128, CH:2 * CH], q_ps[64:128, :])
                                else:
                                    nc.scalar.activation(qpad[c][0:64, 0:CH], q_ps[0:64, :],
                                                         AF.Identity, bias=bq_t[0:64, c:c + 1])
                                    nc.scalar.activation(qpad[c][64:128, CH:2 * CH], q_ps[64:128, :],
                                                         AF.Identity, bias=bq_t[64:128, c:c + 1])

                        # attention: scores^T, exp, Z tree, pair-packed A@V
                        if blvl < 2: continue
                        av_t = [avp.tile([128, 256], F32, name="tl", tag=f"av{c}")
                                for c in range(2)]
                        e_ch = []
                        for ch in range(4):
                            if sc1b:
                                sc_mi = [scp.tile([128, 512], F32, name="tl", tag="sc")
                                         for _ in range(2)]
                                sc_at = lambda mi, lo, hi: sc_mi[mi][:, lo:hi]
                            else:
                                sc_t = scp.tile([128, 1024], F32, name="tl", tag="sc")
                                sc_at = lambda mi, lo, hi: sc_t[:, mi * 512 + lo:mi * 512 + hi]
                            for mi in range(2):
                                m = 2 * ch + mi
                                if rowsc:
                                    for c in range(2):
                                        for hh in range(2):
                                            h = 2 * c + hh
                                            nc.tensor.matmul(
                                                sc_at(mi, h * 128, (h + 1) * 128),
                                                kh_all[64 * hh:64 * hh + 64,
                                                       c * 1024 + m * 128:c * 1024 + (m + 1) * 128],
                                                q_sb[c][64 * hh:64 * hh + 64, :],
                                                start=True, stop=True)
                                else:
                                    for c in range(2):
                                        nc.tensor.matmul(
                                            sc_at(mi, 2 * c * 128, (2 * c + 2) * 128),
                                            kh_all[:, c * 1024 + m * 128:c * 1024 + (m + 1) * 128],
                                            qpad[c][:], start=True, stop=True)
                            e_t = ep.tile([128, 1024], BF16, name="tl", tag="exps")
                            e_ch.append(e_t)
                            if expbig:
                                nc.scalar.activation(e_t[:], sc_t[:], AF.Exp)
                            else:
                                for mi in range(2):
                                    nc.scalar.activation(e_t[:, mi * 512:(mi + 1) * 512],
                                                         sc_at(mi, 0, 512), AF.Exp)
                            if blvl >= 3 and ch in (1, 3):
                                zw = zpl.tile([128, 1024], BF16, name="tl", tag=f"zw{ch}")
                                nc.vector.tensor_add(zw[:], e_ch[ch - 1][:], e_t[:])
                                e_ch[ch] = zw  # keep partial sums, free e tiles
                            for mi in (range(2) if blvl >= 4 else []):
                                m = 2 * ch + mi
                                for c in range(2):
                                    nc.tensor.matmul(
                                        av_t[c][:],
                                        vt_all[:, m * 256 + c * 128:m * 256 + (c + 1) * 128],
                                        e_t[:, mi * 512 + 2 * c * 128:mi * 512 + (2 * c + 2) * 128],
                                        start=(m == 0), stop=(m == NC - 1))
                        if blvl < 3: continue
                        zC = zpl.tile([128, 1024], BF16, name="tl", tag="zC")
                        nc.vector.tensor_add(zC[:], e_ch[1][:], e_ch[3][:])
                        zall = zpl.tile([128, 512], BF16, name="tl", tag="zall")
                        nc.vector.tensor_add(zall[:], zC[:, 0:512], zC[:, 512:1024])
                        z_ps = scp.tile([64, 512], F32, name="tl", tag="sc")
                        nc.tensor.matmul(z_ps[:], ones64b[:], zall[:], start=True, stop=True)
                        # normalize: r_row[0, h*CH:] = 1/Z_h ; broadcast to [128, 256]
                        if blvl < 5: continue
                        r_row = sp_.tile([1, 512], F32, name="tl", tag="rz")
                        nc.vector.tensor_copy(r_row[:], z_ps[0:1, :])
                        nc.vector.reciprocal(r_row[:], r_row[:])
                        b_ps = ps.tile([128, 256], F32, name="tl", tag="ps")
                        for c in range(2):
                            for hh in range(2):
                                h = 2 * c + hh
                                nc.tensor.matmul(b_ps[64 * hh:64 * hh + 64, c * 128:(c + 1) * 128],
                                                 ones_r[:, 0:64], r_row[:, h * 128:(h + 1) * 128],
                                                 start=True, stop=True,
                                                 tile_position=(0, 64 * hh))
                        b_sb = ap_.tile([128, 256], F32, name="tl", tag="bcsb")
                        nc.vector.tensor_copy(b_sb[:], b_ps[:])
                        attn_sb = ap_.tile([128, 256], BF16, name="tl", tag="attn")
                        for c in range(2):
                            nc.vector.tensor_mul(attn_sb[0:64, c * 128:(c + 1) * 128],
                                                 av_t[c][0:64, 0:128],
                                                 b_sb[0:64, c * 128:(c + 1) * 128])
                            nc.vector.tensor_mul(attn_sb[64:128, c * 128:(c + 1) * 128],
                                                 av_t[c][64:128, 128:256],
                                                 b_sb[64:128, c * 128:(c + 1) * 128])

                        # merge
                        if blvl < 6: continue
                        msg_sb = [ap_.tile([128, CH], BF16, name="tl", tag=f"ms{c}") for c in range(2)]
                        for c in range(2):
                            m_ps = ps.tile([128, CH], F32, name="tl", tag="ps")
                            for cc in range(2):
                                nc.tensor.matmul(m_ps[:],
                                                 mg_t[:, cc * 256 + c * 128:cc * 256 + (c + 1) * 128],
                                                 attn_sb[:, cc * 128:(cc + 1) * 128],
                                                 start=(cc == 0), stop=(cc == 1))
                            if zb:
                                nc.vector.tensor_copy(msg_sb[c][:], m_ps[:])
                            else:
                                nc.scalar.activation(msg_sb[c][:], m_ps[:], AF.Identity, bias=mb_t[:, c:c + 1])

                        # mlp1 (relu) on concat([x, msg])
                        if blvl < 7: continue
                        h_in = [dstb[:, xi, 0, :], dstb[:, xi, 1, :], msg_sb[0][:], msg_sb[1][:]]
                        h1_sb = [ap_.tile([128, CH], BF16, name="tl", tag=f"h1{c}") for c in range(4)]
                        for c in range(4):
                            h_ps = ps.tile([128, CH], F32, name="tl", tag="ps")
                            for cc in range(4):
                                nc.tensor.matmul(h_ps[:],
                                                 w1_t[:, cc * 512 + c * 128:cc * 512 + (c + 1) * 128],
                                                 h_in[cc], start=(cc == 0), stop=(cc == 3))
                            if zb:
                                nc.vector.tensor_relu(h1_sb[c][:], h_ps[:])
                            else:
                                nc.scalar.activation(h1_sb[c][:], h_ps[:], AF.Relu, bias=b1_t[:, c:c + 1])

                        # mlp2 -> delta accumulation
                        if blvl < 8: continue
                        for c in range(2):
                            d_ps = ps.tile([128, CH], F32, name="tl", tag="ps")
                            for cc in range(4):
                                nc.tensor.matmul(d_ps[:],
                                                 w2_t[:, cc * 256 + c * 128:cc * 256 + (c + 1) * 128],
                                                 h1_sb[cc][:], start=(cc == 0), stop=(cc == 3))
                            if first_delta[xi]:
                                if zb:
                                    nc.vector.tensor_copy(dlt[:, xi, c, :], d_ps[:])
                                else:
                                    nc.scalar.activation(dlt[:, xi, c, :], d_ps[:], AF.Identity,
                                                         bias=b2_t[:, c:c + 1])
                            else:
                                if zb:
                                    nc.vector.tensor_add(dlt[:, xi, c, :], dlt[:, xi, c, :], d_ps[:])
                                else:
                                    tmp = ap_.tile([128, CH], F32, name="tl", tag="dtmp")
                                    nc.scalar.activation(tmp[:], d_ps[:], AF.Identity, bias=b2_t[:, c:c + 1])
                                    nc.vector.tensor_add(dlt[:, xi, c, :], dlt[:, xi, c, :], tmp[:])
                        first_delta[xi] = False

                # ---- stage C: residual + LayerNorm per tensor
                g_row = sp_.tile([1, 256], F32, name="tl", tag="grow")
                b_col = bp.tile([128, 2], F32, name="tl", tag="lnb")
                for t in (range(4) if "C" in stages else []):
                    nc.sync.dma_start(g_row[:], lng[i, t][None, :])
                    nc.sync.dma_start(b_col[:], lnb[i, t].rearrange("(a p) -> p a", p=128))
                    # X layout: [xn_c0 | sq_c0 | xn_c1 | sq_c1], each 128 cols
                    X = ap_.tile([128, 512], F32, name="tl", tag="lnx")
                    Xv = X[:].rearrange("p (c k f) -> p c k f", c=2, k=2)
                    nc.vector.tensor_add(Xv[:, :, 0, :], dst[:, t, :, :], dlt[:, t, :, :])
                    nc.vector.tensor_mul(Xv[:, :, 1, :], Xv[:, :, 0, :], Xv[:, :, 0, :])
                    s_ps = ps.tile([128, 2 * CH], F32, name="tl", tag="ps")
                    for c in range(2):
                        nc.tensor.matmul(s_ps[0:64, :], ones64[:], X[:, c * 256:(c + 1) * 256],
                                         start=(c == 0), stop=(c == 1))
                    muv = sp_.tile([1, 2 * CH], F32, name="tl", tag="muv")
                    nc.vector.tensor_scalar_mul(muv[:], s_ps[0:1, :], 1.0 / 256)
                    mu, msq = muv[:, 0:CH], muv[:, CH:2 * CH]
                    var = sp_.tile([1, CH], F32, name="tl", tag="var")
                    nc.vector.tensor_mul(var[:], mu, mu)
                    nc.vector.tensor_sub(var[:], msq, var[:])
                    # rsqrt(var+eps) = exp(-0.5*ln(var+eps)): keeps ACT in the
                    # exp/ln table set (no Sqrt table switch mid-kernel)
                    lnv = sp_.tile([1, CH], F32, name="tl", tag="lnv")
                    nc.scalar.activation(lnv[:], var[:], AF.Ln, bias=eps_c[:])
                    rs = sp_.tile([1, CH], F32, name="tl", tag="rs")
                    nc.scalar.activation(rs[:], lnv[:], AF.Exp, scale=-0.5)
                    mu_ps = ps.tile([128, CH], F32, name="tl", tag="ps")
                    nc.tensor.matmul(mu_ps[:], ones_r[:], mu, start=True, stop=True)
                    b2_ps = ps.tile([128, 2 * CH], F32, name="tl", tag="ps")
                    for c in range(2):
                        nc.tensor.matmul(b2_ps[:, c * 128:(c + 1) * 128],
                                         g_row[:, c * 128:(c + 1) * 128], rs[:],
                                         start=True, stop=True)
                    t1 = ap_.tile([128, 2 * CH], F32, name="tl", tag="t1")
                    for c in range(2):
                        nc.vector.tensor_sub(t1[:, c * 128:(c + 1) * 128],
                                             X[:, c * 256:c * 256 + 128], mu_ps[:])
                    nc.vector.tensor_mul(t1[:], t1[:], b2_ps[:])
                    for c in range(2):
                        nc.vector.tensor_scalar_add(dst[:, t, c, :], t1[:, c * 128:(c + 1) * 128],
                                                    b_col[:, c:c + 1])
                    nc.vector.tensor_copy(dstb[:, t, :, :], dst[:, t, :, :])

            # ---- epilogue: out[m] = (1/32) qvec^T kmat[:, m]
            s1 = sp_.tile([128, 2], F32, name="tl", tag="s1")
            for c in range(2):
                nc.vector.reduce_sum(s1[:, c:c + 1], dst[:, 1, c, :], axis=mybir.AxisListType.X)
                nc.sync.dma_start(ag2in[c], s1[:, c:c + 1])
            nc.gpsimd.collective_compute(
                "AllGather", mybir.AluOpType.bypass, replica_groups=rg,
                ins=[ag2in.ap().opt()], outs=[ag2out.ap().opt()])
            d1b = sp_.tile([128, 2], F32, name="tl", tag="d1b")
            gath = sp_.tile([128, NC], F32, name="tl", tag="gath")
            for c in range(2):
                nc.sync.dma_start(gath[:], ag2out.ap().rearrange("r c p o -> c p (r o)")[c])
                nc.vector.reduce_sum(d1b[:, c:c + 1], gath[:], axis=mybir.AxisListType.X)

            wq5 = [cpool.tile([128, 256], F32, name="tl", tag=f"wq5{k}") for k in range(2)]
            wk5 = [cpool.tile([128, 256], F32, name="tl", tag=f"wk5{k}") for k in range(2)]
            for k in range(2):
                nc.sync.dma_start(wq5[k][:], w5T[0, k * 128:(k + 1) * 128, :])
                nc.sync.dma_start(wk5[k][:], w5T[1, k * 128:(k + 1) * 128, :])
            b5 = bp.tile([128, 4], F32, name="tl", tag="b5")
            nc.sync.dma_start(b5[:], pb5.rearrange("t (a p) -> p (t a)", p=128))
            qv = sp_.tile([128, 2], F32, name="tl", tag="qv")
            for c in range(2):
                q_ps = ps.tile([128, CH], F32, name="tl", tag="ps")
                for cc in range(2):
                    nc.tensor.matmul(q_ps[:, 0:1], wq5[cc][:, c * 128:(c + 1) * 128],
                                     d1b[:, cc:cc + 1], start=(cc == 0), stop=(cc == 1))
                nc.scalar.activation(qv[:, c:c + 1], q_ps[:, 0:1], AF.Identity,
                                     bias=b5[:, c:c + 1], scale=1.0 / N)
            km = [ap_.tile([128, CH], F32, name="tl", tag=f"km{c}") for c in range(2)]
            for c in range(2):
                k_ps = ps.tile([128, CH], F32, name="tl", tag="ps")
                for cc in range(2):
                    nc.tensor.matmul(k_ps[:], wk5[cc][:, c * 128:(c + 1) * 128],
                                     dst[:, 0, cc, :], start=(cc == 0), stop=(cc == 1))
                nc.scalar.activation(km[c][:], k_ps[:], AF.Identity, bias=b5[:, 2 + c:3 + c])
            o_ps = ps.tile([128, CH], F32, name="tl", tag="ps")
            for c in range(2):
                nc.vector.tensor_scalar_mul(km[c][:], km[c][:], qv[:, c:c + 1])
                nc.tensor.matmul(o_ps[0:64, :], ones64[:], km[c][:],
                                 start=(c == 0), stop=(c == 1))
            o_sb = sp_.tile([1, CH], F32, name="tl", tag="osb")
            nc.scalar.activation(o_sb[:], o_ps[0:1, :], AF.Copy, scale=1.0 / 32)
            nc.sync.dma_start(out_d[:], o_sb[:])

    nc.compile()
    return nc


def prep_inputs(inputs):
    inp = {k: np.ascontiguousarray(np.asarray(v)) for k, v in inputs.items()}
    pw, pb = inp['proj_w'].astype(np.float32), inp['proj_b'].astype(np.float32)
    mw, mb = inp['merge_w'].astype(np.float32), inp['merge_b'].astype(np.float32)
    w1, b1 = inp['mlp_w1'].astype(np.float32), inp['mlp_b1'].astype(np.float32)
    w2, b2 = inp['mlp_w2'].astype(np.float32), inp['mlp_b2'].astype(np.float32)
    ng, nb = inp['norm_g'].astype(np.float32), inp['norm_b'].astype(np.float32)

    wqkvT = np.empty((5, 3, 3, 256, 256), np.float32)
    mergeT = np.empty((5, 3, 256, 256), np.float32)
    w1T = np.empty((5, 3, 512, 512), np.float32)
    w2T = np.empty((5, 3, 512, 256), np.float32)
    pbq = np.empty((5, 3, 256), np.float32)
    pbk = np.empty((5, 3, 256), np.float32)
    pbv = np.empty((5, 3, 256), np.float32)
    for i in range(5):
        for br in range(3):
            for j in range(3):
                wqkvT[i, br, j] = pw[br, i, j][PERM].T
            wqkvT[i, br, 0] *= 0.125
            pbq[i, br] = pb[br, i, 0][PERM] * 0.125
            pbk[i, br] = pb[br, i, 1][PERM]
            pbv[i, br] = pb[br, i, 2][PERM]
            mergeT[i, br] = mw[br, i][:, PERM].T
            w1T[i, br] = w1[br, i].T
            w2T[i, br] = w2[br, i].T
    mbbv = np.transpose(mb[:, :5], (1, 0, 2)).astype(np.float32).copy()
    b1bv = np.transpose(b1[:, :5], (1, 0, 2)).astype(np.float32).copy()
    b2bv = np.transpose(b2[:, :5], (1, 0, 2)).astype(np.float32).copy()
    lngv = np.transpose(ng[:, :5], (1, 0, 2)).astype(np.float32).copy()
    lnbv = np.transpose(nb[:, :5], (1, 0, 2)).astype(np.float32).copy()
    w5T = np.stack([pw[0, 5, 0].T, pw[0, 5, 1].T]).astype(np.float32)
    pb5 = np.stack([pb[0, 5, 0], pb[0, 5, 1]]).astype(np.float32)

    desc = np.stack([inp[f'desc{t}'][0] for t in range(4)]).astype(np.float32)  # [4,256,N]
    bf = mybir.dt.np(mybir.dt.bfloat16)
    wqkvT = wqkvT.astype(bf); mergeT = mergeT.astype(bf)
    w1T = w1T.astype(bf); w2T = w2T.astype(bf)
    shared = dict(wqkvT=wqkvT, mergeT=mergeT, w1T=w1T, w2T=w2T, pbq=pbq, pbk=pbk,
                  pbv=pbv, mbb=mbbv, b1b=b1bv, b2b=b2bv, lng=lngv, lnb=lnbv,
                  w5T=w5T, pb5=pb5)
    in_maps = []
    for j in range(NC):
        xcj = desc[:, :, j * CH:(j + 1) * CH].reshape(4, 2, 128, CH)
        in_maps.append({"xc": np.ascontiguousarray(xcj), **shared})
    return in_maps


def kernel(**inputs):
    zb = all(not np.asarray(inputs[k]).any() for k in
             ("proj_b", "merge_b", "mlp_b1", "mlp_b2"))
    key = f"nc{zb}"
    if key not in _cache:
        _cache[key] = build_kernel(zb=zb)
    nc = _cache[key]
    in_maps = prep_inputs(inputs)
    res = run_bass_kernel_spmd(nc, in_maps, core_ids=list(range(NC)))
    out = np.concatenate([res.results[j]["out"][0] for j in range(NC)])
    mask = np.asarray(inputs["unreachable"]).any(axis=0)
    out = np.where(mask, np.float32(-1e9), out.astype(np.float32))
    return out


# revision 42
# speedup vs baseline: 1.1070x; 1.1070x over previous
"""Trainium2 Bass kernel for nn_AttentionalGNN (8-core SPMD, sequence-sharded).

Strategy:
  - Shard the N=1024 node axis across 8 cores (128 queries each). All of
    attention/merge/MLP/LayerNorm is local per position; only k/v need the full
    sequence, shared via fused AllGathers (2 per layer, software-pipelined so
    the second AG overlaps the first half's attention compute).
  - Heads are made contiguous by permuting projection output channels host-side
    (orig channel d = 4a+b -> row b*64+a), with the merge weight columns
    permuted to match.
  - Scores are computed per-head with K=64 row-tiled matmuls (two heads run
    concurrently in the PE array's upper/lower row groups); exp is batched as
    one activation over a 2-bank PSUM score buffer; the softmax denominator is
    a bf16 tree-reduction on DVE plus one ones-matmul partition reduce.
  - The final output only uses the layer-5 scores of prop(0,5,d1,d0), reduced
    over heads and queries: out[m] = (1/32) * (Wq5 @ mean_n d1)^T (Wk5 @ d0)[:,m].
    So layer 5 collapses to two projections + a tiny cross-core reduce.
"""
import numpy as np

import concourse.bass as bass
import concourse.bacc as bacc
import concourse.mybir as mybir
import concourse.tile as tile
from concourse.bass_utils import run_bass_kernel_spmd

D, N, H, DH = 256, 1024, 4, 64
NC = 8
CH = N // NC  # 128 positions per core
F32 = mybir.dt.float32
BF16 = mybir.dt.bfloat16
AF = mybir.ActivationFunctionType

PROPS_SELF = [(0, 0, 0), (0, 1, 1), (1, 2, 2), (2, 3, 3)]
PROPS_CROSS = [(0, 0, 1), (0, 1, 0), (1, 2, 1), (1, 1, 2), (2, 0, 3), (2, 3, 0)]
NAMES5 = ['self', 'cross', 'self', 'cross', 'self']
PERM = np.array([4 * (r % 64) + r // 64 for r in range(256)])

_cache = {}


def _props(i):
    return PROPS_CROSS if NAMES5[i] == 'cross' else PROPS_SELF


def build_kernel(trace_scopes=False, n_layers=5, stages="ABC", blvl=9, reps=1,
                 zb=True, agsplit=2, expbig=False, rowsc=False, sc1b=True):
    if sc1b:
        expbig = False
    nc = bacc.Bacc("TRN2", target_bir_lowering=False, debug=False, num_devices=NC)

    # ---- I/O ----
    xc = nc.dram_tensor("xc", [4, 2, 128, CH], F32, kind="ExternalInput")
    wqkvT = nc.dram_tensor("wqkvT", [5, 3, 3, 256, 256], BF16, kind="ExternalInput")
    mergeT = nc.dram_tensor("mergeT", [5, 3, 256, 256], BF16, kind="ExternalInput")
    w1T = nc.dram_tensor("w1T", [5, 3, 512, 512], BF16, kind="ExternalInput")
    w2T = nc.dram_tensor("w2T", [5, 3, 512, 256], BF16, kind="ExternalInput")
    pbq = nc.dram_tensor("pbq", [5, 3, 256], F32, kind="ExternalInput")
    pbk = nc.dram_tensor("pbk", [5, 3, 256], F32, kind="ExternalInput")
    pbv = nc.dram_tensor("pbv", [5, 3, 256], F32, kind="ExternalInput")
    mbb = nc.dram_tensor("mbb", [5, 3, 256], F32, kind="ExternalInput")
    b1b = nc.dram_tensor("b1b", [5, 3, 512], F32, kind="ExternalInput")
    b2b = nc.dram_tensor("b2b", [5, 3, 256], F32, kind="ExternalInput")
    lng = nc.dram_tensor("lng", [5, 4, 256], F32, kind="ExternalInput")
    lnb = nc.dram_tensor("lnb", [5, 4, 256], F32, kind="ExternalInput")
    w5T = nc.dram_tensor("w5T", [2, 256, 256], F32, kind="ExternalInput")  # [qT,kT]
    pb5 = nc.dram_tensor("pb5", [2, 256], F32, kind="ExternalInput")
    out_d = nc.dram_tensor("out", [1, CH], F32, kind="ExternalOutput")

    # per-(layer, group) collective buffers (compile-time static)
    # chunk layout per prop (65536 elems): [0:16384] k c=0 rows(p)=dims0..127,
    # [16384:32768] k c=1, [32768:65536] vT [128 keys, 256 dims]
    groups = []  # groups[i] = list of (props_slice, agin, agout)
    for i in range(5):
        props = _props(i)
        if agsplit == 2:
            half = (len(props) + 1) // 2
        elif agsplit >= 3:
            half = 2
        else:
            half = len(props)
        gs = []
        for g, lo in enumerate(range(0, len(props), half)):
            grp = props[lo:lo + half]
            agin = nc.dram_tensor(f"agin{i}_{g}", [len(grp), 65536], BF16)
            agout = nc.dram_tensor(f"agout{i}_{g}", [NC, len(grp), 65536], BF16,
                                   addr_space="Shared")
            gs.append((lo, grp, agin, agout))
        groups.append(gs)
    ag2in = nc.dram_tensor("ag2in", [2, 128, 1], F32)
    ag2out = nc.dram_tensor("ag2out", [NC, 2, 128, 1], F32, addr_space="Shared")

    # persistent SBUF state
    dst = nc.alloc_sbuf_tensor("dstate", [128, 4, 2, CH], F32)   # d[t] ctile c
    dlt = nc.alloc_sbuf_tensor("delta", [128, 4, 2, CH], F32)
    dstb = nc.alloc_sbuf_tensor("dstateb", [128, 4, 2, CH], BF16)

    rg = [list(range(NC))]

    from contextlib import ExitStack
    with ExitStack() as es:
        tc = es.enter_context(tile.TileContext(nc))
        cpool = es.enter_context(tc.tile_pool(name="const", bufs=1))
        qkvp = es.enter_context(tc.tile_pool(name="qkv", bufs=4))
        mgp = es.enter_context(tc.tile_pool(name="mg", bufs=2))
        w1p = es.enter_context(tc.tile_pool(name="w1", bufs=2))
        w2p = es.enter_context(tc.tile_pool(name="w2", bufs=2))
        bp = es.enter_context(tc.tile_pool(name="bias", bufs=3))
        khp = es.enter_context(tc.tile_pool(name="kh", bufs=3))
        vtp = es.enter_context(tc.tile_pool(name="vt", bufs=3))
        ap_ = es.enter_context(tc.tile_pool(name="act", bufs=4))
        ep = es.enter_context(tc.tile_pool(name="exps", bufs=4))
        zpl = es.enter_context(tc.tile_pool(name="zs", bufs=2))
        sp_ = es.enter_context(tc.tile_pool(name="small", bufs=4))
        ps = es.enter_context(tc.tile_pool(name="ps", bufs=2, space="PSUM"))
        scp = es.enter_context(tc.tile_pool(name="sc", bufs=2, space="PSUM"))
        avp = es.enter_context(tc.tile_pool(name="av", bufs=2, space="PSUM"))
        if True:
            ones_c = cpool.tile([128, 1], F32, name="tl", tag="ones_c")
            nc.gpsimd.memset(ones_c[:], 1.0)
            ones_r = cpool.tile([1, 128], F32, name="tl", tag="ones_r")
            nc.gpsimd.memset(ones_r[:], 1.0)
            eps_c = cpool.tile([1, 1], F32, name="tl", tag="eps_c")
            nc.gpsimd.memset(eps_c[:], 1e-5)
            ones64 = cpool.tile([128, 64], F32, name="tl", tag="ones64")
            nc.gpsimd.memset(ones64[:], 1.0)
            ones64b = cpool.tile([128, 64], BF16, name="tl", tag="ones64b")
            nc.gpsimd.memset(ones64b[:], 1.0)
            qpad = None
            if not rowsc:
                qpad = [cpool.tile([128, 256], BF16, name="tl", tag=f"qpad{c}") for c in range(2)]
                for c in range(2):
                    nc.gpsimd.memset(qpad[c][:], 0.0)

            # load descriptor chunks into d-state
            for t in range(4):
                for c in range(2):
                    nc.sync.dma_start(dst[:, t, c, :], xc[t, c])
                nc.vector.tensor_copy(dstb[:, t, :, :], dst[:, t, :, :])

            def load_qkv(i, br):
                t_ = qkvp.tile([128, 1536], BF16, name="tl", tag="qkv")
                nc.sync.dma_start(t_[:].rearrange("p (a f) -> p a f", a=6),
                                  wqkvT[i, br].rearrange("j (k p) f -> p (j k) f", p=128))
                return t_

            def load_wide(pool, src_ap, k, f, tag):
                t_ = pool.tile([128, k * f], BF16, name="tl", tag=tag)
                nc.sync.dma_start(t_[:].rearrange("p (k f) -> p k f", k=k),
                                  src_ap.rearrange("(k p) f -> p k f", p=128))
                return t_

            def load_bias(src_ap, n, tag):
                t_ = bp.tile([128, n // 128], F32, name="tl", tag=tag)
                nc.sync.dma_start(t_[:], src_ap.rearrange("(a p) -> p a", p=128))
                return t_

            for i in [li for _r in range(reps) for li in range(n_layers)]:
                props = _props(i)
                qkv_tiles = {}

                # ---- stage A: per group produce local k/v chunks, then AllGather
                for lo, grp, agin, agout in (groups[i] if "A" in stages else []):
                    cur_br = None
                    bk_t = bv_b = None
                    for gi, (br, xi, si) in enumerate(grp):
                        if br not in qkv_tiles:
                            qkv_tiles[br] = load_qkv(i, br)
                        qkv = qkv_tiles[br]
                        if br != cur_br and not zb:
                            cur_br = br
                            bk_t = load_bias(pbk[i, br], 256, "bk")
                            bvr = sp_.tile([1, 256], F32, name="tl", tag="bvr")
                            nc.sync.dma_start(bvr[:], pbv[i, br][None, :])
                            bv_ps = ps.tile([128, 256], F32, name="tl", tag="ps")
                            nc.tensor.matmul(bv_ps[:], ones_r[:], bvr[:], start=True, stop=True)
                            bv_b = sp_.tile([128, 256], F32, name="tl", tag="bvb")
                            nc.vector.tensor_copy(bv_b[:], bv_ps[:])
                        # k chunk: [256, CH] as 2 ctiles
                        for c in range(2):
                            kc_ps = ps.tile([128, CH], F32, name="tl", tag="ps")
                            for cc in range(2):
                                nc.tensor.matmul(kc_ps[:],
                                                 qkv[:, 512 + cc * 256 + c * 128:512 + cc * 256 + (c + 1) * 128],
                                                 dstb[:, si, cc, :], start=(cc == 0), stop=(cc == 1))
                            kc_sb = ap_.tile([128, CH], BF16, name="tl", tag="kcsb")
                            if zb:
                                nc.vector.tensor_copy(kc_sb[:], kc_ps[:])
                            else:
                                nc.scalar.activation(kc_sb[:], kc_ps[:], AF.Identity, bias=bk_t[:, c:c + 1])
                            nc.sync.dma_start(
                                agin[gi, c * 16384:(c + 1) * 16384].rearrange("(p f) -> p f", p=128),
                                kc_sb[:])
                        # vT chunk: [CH, 256]
                        vt_ps = ps.tile([128, 256], F32, name="tl", tag="ps")
                        for cc in range(2):
                            nc.tensor.matmul(vt_ps[:], dstb[:, si, cc, :],
                                             qkv[:, 1024 + cc * 256:1024 + (cc + 1) * 256],
                                             start=(cc == 0), stop=(cc == 1))
                        vt_sb = ap_.tile([128, 256], BF16, name="tl", tag="vtsb")
                        if zb:
                            nc.vector.tensor_copy(vt_sb[:], vt_ps[:])
                        else:
                            nc.vector.tensor_add(vt_sb[:], vt_ps[:], bv_b[:])
                        nc.sync.dma_start(
                            agin[gi, 32768:65536].rearrange("(p f) -> p f", p=128),
                            vt_sb[:])
                    nc.gpsimd.collective_compute(
                        "AllGather", mybir.AluOpType.bypass, replica_groups=rg,
                        ins=[agin.ap().opt()], outs=[agout.ap().opt()])

                # ---- stage B: per prop attention + merge + MLP on local queries
                first_delta = {t: True for t in range(4)}
                cur_br = None
                wq_t = mg_t = w1_t = w2_t = None
                bq_t = mb_t = b1_t = b2_t = None
                for lo, grp, agin, agout in (groups[i] if "B" in stages else []):
                    for gi, (br, xi, si) in enumerate(grp):
                        if br != cur_br:
                            cur_br = br
                            if br not in qkv_tiles:
                                qkv_tiles[br] = load_qkv(i, br)
                            qkv = qkv_tiles[br]
                            mg_t = load_wide(mgp, mergeT[i, br], 2, 256, "mg")
                            w1_t = load_wide(w1p, w1T[i, br], 4, 512, "w1")
                            w2_t = load_wide(w2p, w2T[i, br], 4, 256, "w2")
                            if not zb:
                                bq_t = load_bias(pbq[i, br], 256, "bq")
                                mb_t = load_bias(mbb[i, br], 256, "mb")
                                b1_t = load_bias(b1b[i, br], 512, "b1")
                                b2_t = load_bias(b2b[i, br], 256, "b2")

                        # gathered k: [128, c(2) x r(8) x 128] ; vT: [128, m(8) x 256]
                        kh_all = khp.tile([128, 2048], BF16, name="tl", tag="kh")
                        for c in range(2):
                            nc.sync.dma_start(
                                kh_all[:, c * 1024:(c + 1) * 1024].rearrange("p (r f) -> p r f", r=NC),
                                agout[:, gi, c * 16384:(c + 1) * 16384].rearrange("r (p f) -> p r f", p=128))
                        vt_all = vtp.tile([128, 2048], BF16, name="tl", tag="vt")
                        nc.sync.dma_start(
                            vt_all[:].rearrange("p (m f) -> p m f", m=NC),
                            agout[:, gi, 32768:65536].rearrange("m (p f) -> p m f", p=128))

                        # q: [256, CH] scaled by 1/8 (folded in weights host-side)
                        q_sb = [sp_.tile([128, CH], BF16, name="tl", tag=f"qsb{c}") for c in range(2)] \
                            if rowsc else None
                        for c in range(2):
                            q_ps = ps.tile([128, CH], F32, name="tl", tag="ps")
                            for cc in range(2):
                                nc.tensor.matmul(q_ps[:],
                                                 qkv[:, cc * 256 + c * 128:cc * 256 + (c + 1) * 128],
                                                 dstb[:, xi, cc, :], start=(cc == 0), stop=(cc == 1))
                            if rowsc:
                                if zb:
                                    nc.vector.tensor_copy(q_sb[c][:], q_ps[:])
                                else:
                                    nc.scalar.activation(q_sb[c][:], q_ps[:], AF.Identity,
                                                         bias=bq_t[:, c:c + 1])
                            else:
                                if zb:
                                    nc.vector.tensor_copy(qpad[c][0:64, 0:CH], q_ps[0:64, :])
                                    nc.vector.tensor_copy(qpad[c][64:128, CH:2 * CH], q_ps[64:128, :])
                                else:
                                    nc.scalar.activation(qpad[c][0:64, 0:CH], q_ps[0:64, :],
                                                         AF.Identity, bias=bq_t[0:64, c:c + 1])
                                    nc.scalar.activation(qpad[c][64:128, CH:2 * CH], q_ps[64:128, :],
                                                         AF.Identity, bias=bq_t[64:128, c:c + 1])

                        # attention: scores^T, exp, Z tree, pair-packed A@V
                        if blvl < 2: continue
                        av_t = [avp.tile([128, 256], F32, name="tl", tag=f"av{c}")
                                for c in range(2)]
                        e_ch = []
                        for ch in range(4):
                            if sc1b:
                                sc_mi = [scp.tile([128, 512], F32, name="tl", tag="sc")
                                         for _ in range(2)]
                                sc_at = lambda mi, lo, hi: sc_mi[mi][:, lo:hi]
                            else:
                                sc_t = scp.tile([128, 1024], F32, name="tl", tag="sc")
                                sc_at = lambda mi, lo, hi: sc_t[:, mi * 512 + lo:mi * 512 + hi]
                            for mi in range(2):
                                m = 2 * ch + mi
                                if rowsc:
                                    for c in range(2):
                                        for hh in range(2):
                                            h = 2 * c + hh
                                            nc.tensor.matmul(
                                                sc_at(mi, h * 128, (h + 1) * 128),
                                                kh_all[64 * hh:64 * hh + 64,
                                                       c * 1024 + m * 128:c * 1024 + (m + 1) * 128],
                                                q_sb[c][64 * hh:64 * hh + 64, :],
                                                start=True, stop=True)
                                else:
                                    for c in range(2):
                                        nc.tensor.matmul(
                                            sc_at(mi, 2 * c * 128, (2 * c + 2) * 128),
                                            kh_all[:, c * 1024 + m * 128:c * 1024 + (m + 1) * 128],
                                            qpad[c][:], start=True, stop=True)
                            e_t = ep.tile([128, 1024], BF16, name="tl", tag="exps")
                            e_ch.append(e_t)
                            if expbig:
                                nc.scalar.activation(e_t[:], sc_t[:], AF.Exp)
                            else:
                                for mi in range(2):
                                    nc.scalar.activation(e_t[:, mi * 512:(mi + 1) * 512],
                                                         sc_at(mi, 0, 512), AF.Exp)
                            if blvl >= 3 and ch in (1, 3):
                                zw = zpl.tile([128, 1024], BF16, name="tl", tag=f"zw{ch}")
                                nc.vector.tensor_add(zw[:], e_ch[ch - 1][:], e_t[:])
                                e_ch[ch] = zw  # keep partial sums, free e tiles
                            for mi in (range(2) if blvl >= 4 else []):
                                m = 2 * ch + mi
                                for c in range(2):
                                    nc.tensor.matmul(
                                        av_t[c][:],
                                        vt_all[:, m * 256 + c * 128:m * 256 + (c + 1) * 128],
                                        e_t[:, mi * 512 + 2 * c * 128:mi * 512 + (2 * c + 2) * 128],
                                        start=(m == 0), stop=(m == NC - 1))
                        if blvl < 3: continue
                        zC = zpl.tile([128, 1024], BF16, name="tl", tag="zC")
                        nc.vector.tensor_add(zC[:], e_ch[1][:], e_ch[3][:])
                        zall = zpl.tile([128, 512], BF16, name="tl", tag="zall")
                        nc.vector.tensor_add(zall[:], zC[:, 0:512], zC[:, 512:1024])
                        z_ps = scp.tile([64, 512], F32, name="tl", tag="sc")
                        nc.tensor.matmul(z_ps[:], ones64b[:], zall[:], start=True, stop=True)
                        # normalize: r_row[0, h*CH:] = 1/Z_h ; broadcast to [128, 256]
                        if blvl < 5: continue
                        r_row = sp_.tile([1, 512], F32, name="tl", tag="rz")
                        nc.vector.tensor_copy(r_row[:], z_ps[0:1, :])
                        nc.vector.reciprocal(r_row[:], r_row[:])
                        b_ps = ps.tile([128, 256], F32, name="tl", tag="ps")
                        for c in range(2):
                            for hh in range(2):
                                h = 2 * c + hh
                                nc.tensor.matmul(b_ps[64 * hh:64 * hh + 64, c * 128:(c + 1) * 128],
                                                 ones_r[:, 0:64], r_row[:, h * 128:(h + 1) * 128],
                                                 start=True, stop=True,
                                                 tile_position=(0, 64 * hh))
                        b_sb = ap_.tile([128, 256], F32, name="tl", tag="bcsb")
                        nc.vector.tensor_copy(b_sb[:], b_ps[:])
                        attn_sb = ap_.tile([128, 256], BF16, name="tl", tag="attn")
                        for c in range(2):
                            nc.vector.tensor_mul(attn_sb[0:64, c * 128:(c + 1) * 128],
                                                 av_t[c][0:64, 0:128],
                                                 b_sb[0:64, c * 128:(c + 1) * 128])
                            nc.vector.tensor_mul(attn_sb[64:128, c * 128:(c + 1) * 128],
                                                 av_t[c][64:128, 128:256],
                                                 b_sb[64:128, c * 128:(c + 1) * 128])

                        # merge
                        if blvl < 6: continue
                        msg_sb = [ap_.tile([128, CH], BF16, name="tl", tag=f"ms{c}") for c in range(2)]
                        for c in range(2):
                            m_ps = ps.tile([128, CH], F32, name="tl", tag="ps")
                            for cc in range(2):
                                nc.tensor.matmul(m_ps[:],
                                                 mg_t[:, cc * 256 + c * 128:cc * 256 + (c + 1) * 128],
                                                 attn_sb[:, cc * 128:(cc + 1) * 128],
                                                 start=(cc == 0), stop=(cc == 1))
                            if zb:
                                nc.vector.tensor_copy(msg_sb[c][:], m_ps[:])
                            else:
                                nc.scalar.activation(msg_sb[c][:], m_ps[:], AF.Identity, bias=mb_t[:, c:c + 1])

                        # mlp1 (relu) on concat([x, msg])
                        if blvl < 7: continue
                        h_in = [dstb[:, xi, 0, :], dstb[:, xi, 1, :], msg_sb[0][:], msg_sb[1][:]]
                        h1_sb = [ap_.tile([128, CH], BF16, name="tl", tag=f"h1{c}") for c in range(4)]
                        for c in range(4):
                            h_ps = ps.tile([128, CH], F32, name="tl", tag="ps")
                            for cc in range(4):
                                nc.tensor.matmul(h_ps[:],
                                                 w1_t[:, cc * 512 + c * 128:cc * 512 + (c + 1) * 128],
                                                 h_in[cc], start=(cc == 0), stop=(cc == 3))
                            if zb:
                                nc.vector.tensor_relu(h1_sb[c][:], h_ps[:])
                            else:
                                nc.scalar.activation(h1_sb[c][:], h_ps[:], AF.Relu, bias=b1_t[:, c:c + 1])

                        # mlp2 -> delta accumulation
                        if blvl < 8: continue
                        for c in range(2):
                            d_ps = ps.tile([128, CH], F32, name="tl", tag="ps")
                            for cc in range(4):
                                nc.tensor.matmul(d_ps[:],
                                                 w2_t[:, cc * 256 + c * 128:cc * 256 + (c + 1) * 128],
                                                 h1_sb[cc][:], start=(cc == 0), stop=(cc == 3))
                            if first_delta[xi]:
                                if zb:
                                    nc.vector.tensor_copy(dlt[:, xi, c, :], d_ps[:])
                                else:
                                    nc.scalar.activation(dlt[:, xi, c, :], d_ps[:], AF.Identity,
                                                         bias=b2_t[:, c:c + 1])
                            else:
                                if zb:
                                    nc.vector.tensor_add(dlt[:, xi, c, :], dlt[:, xi, c, :], d_ps[:])
                                else:
                                    tmp = ap_.tile([128, CH], F32, name="tl", tag="dtmp")
                                    nc.scalar.activation(tmp[:], d_ps[:], AF.Identity, bias=b2_t[:, c:c + 1])
                                    nc.vector.tensor_add(dlt[:, xi, c, :], dlt[:, xi, c, :], tmp[:])
                        first_delta[xi] = False

                # ---- stage C: residual + LayerNorm per tensor
                g_row = sp_.tile([1, 256], F32, name="tl", tag="grow")
                b_col = bp.tile([128, 2], F32, name="tl", tag="lnb")
                for t in (range(4) if "C" in stages else []):
                    nc.sync.dma_start(g_row[:], lng[i, t][None, :])
                    nc.sync.dma_start(b_col[:], lnb[i, t].rearrange("(a p) -> p a", p=128))
                    # X layout: [xn_c0 | sq_c0 | xn_c1 | sq_c1], each 128 cols
                    X = ap_.tile([128, 512], F32, name="tl", tag="lnx")
                    Xv = X[:].rearrange("p (c k f) -> p c k f", c=2, k=2)
                    nc.vector.tensor_add(Xv[:, :, 0, :], dst[:, t, :, :], dlt[:, t, :, :])
                    nc.vector.tensor_mul(Xv[:, :, 1, :], Xv[:, :, 0, :], Xv[:, :, 0, :])
                    s_ps = ps.tile([128, 2 * CH], F32, name="tl", tag="ps")
                    for c in range(2):
                        nc.tensor.matmul(s_ps[0:64, :], ones64[:], X[:, c * 256:(c + 1) * 256],
                                         start=(c == 0), stop=(c == 1))
                    muv = sp_.tile([1, 2 * CH], F32, name="tl", tag="muv")
                    nc.vector.tensor_scalar_mul(muv[:], s_ps[0:1, :], 1.0 / 256)
                    mu, msq = muv[:, 0:CH], muv[:, CH:2 * CH]
                    var = sp_.tile([1, CH], F32, name="tl", tag="var")
                    nc.vector.tensor_mul(var[:], mu, mu)
                    nc.vector.tensor_sub(var[:], msq, var[:])
                    # rsqrt(var+eps) = exp(-0.5*ln(var+eps)): keeps ACT in the
                    # exp/ln table set (no Sqrt table switch mid-kernel)
                    lnv = sp_.tile([1, CH], F32, name="tl", tag="lnv")
                    nc.scalar.activation(lnv[:], var[:], AF.Ln, bias=eps_c[:])
                    rs = sp_.tile([1, CH], F32, name="tl", tag="rs")
                    nc.scalar.activation(rs[:], lnv[:], AF.Exp, scale=-0.5)
                    mu_ps = ps.tile([128, CH], F32, name="tl", tag="ps")
                    nc.tensor.matmul(mu_ps[:], ones_r[:], mu, start=True, stop=True)
                    b2_ps = ps.tile([128, 2 * CH], F32, name="tl", tag="ps")
                    for c in range(2):
                        nc.tensor.matmul(b2_ps[:, c * 128:(c + 1) * 128],
                                         g_row[:, c * 128:(c + 1) * 128], rs[:],
                                         start=True, stop=True)
                    t1 = ap_.tile([128, 2 * CH], F32, name="tl", tag="t1")
                    for c in range(2):
                        nc.vector.tensor_sub(t1[:, c * 128:(c + 1) * 128],
                                             X[:, c * 256:c * 256 + 128], mu_ps[:])
                    nc.vector.tensor_mul(t1[:], t1[:], b2_ps[:])
                    for c in range(2):
                        nc.vector.tensor_scalar_add(dst[:, t, c, :], t1[:, c * 128:(c + 1) * 128],
                                                    b_col[:, c:c + 1])
                    nc.vector.tensor_copy(dstb[:, t, :, :], dst[:, t, :, :])

            # ---- epilogue: out[m] = (1/32) qvec^T kmat[:, m]
            s1 = sp_.tile([128, 2], F32, name="tl", tag="s1")
            for c in range(2):
                nc.vector.reduce_sum(s1[:, c:c + 1], dst[:, 1, c, :], axis=mybir.AxisListType.X)
                nc.sync.dma_start(ag2in[c], s1[:, c:c + 1])
            nc.gpsimd.collective_compute(
                "AllGather", mybir.AluOpType.bypass, replica_groups=rg,
                ins=[ag2in.ap().opt()], outs=[ag2out.ap().opt()])
            d1b = sp_.tile([128, 2], F32, name="tl", tag="d1b")
            gath = sp_.tile([128, NC], F32, name="tl", tag="gath")
            for c in range(2):
                nc.sync.dma_start(gath[:], ag2out.ap().rearrange("r c p o -> c p (r o)")[c])
                nc.vector.reduce_sum(d1b[:, c:c + 1], gath[:], axis=mybir.AxisListType.X)

            wq5 = [cpool.tile([128, 256], F32, name="tl", tag=f"wq5{k}") for k in range(2)]
            wk5 = [cpool.tile([128, 256], F32, name="tl", tag=f"wk5{k}") for k in range(2)]
            for k in range(2):
                nc.sync.dma_start(wq5[k][:], w5T[0, k * 128:(k + 1) * 128, :])
                nc.sync.dma_start(wk5[k][:], w5T[1, k * 128:(k + 1) * 128, :])
            b5 = bp.tile([128, 4], F32, name="tl", tag="b5")
            nc.sync.dma_start(b5[:], pb5.rearrange("t (a p) -> p (t a)", p=128))
            qv = sp_.tile([128, 2], F32, name="tl", tag="qv")
            for c in range(2):
                q_ps = ps.tile([128, CH], F32, name="tl", tag="ps")
                for cc in range(2):
                    nc.tensor.matmul(q_ps[:, 0:1], wq5[cc][:, c * 128:(c + 1) * 128],
                                     d1b[:, cc:cc + 1], start=(cc == 0), stop=(cc == 1))
                nc.scalar.activation(qv[:, c:c + 1], q_ps[:, 0:1], AF.Identity,
                                     bias=b5[:, c:c + 1], scale=1.0 / N)
            km = [ap_.tile([128, CH], F32, name="tl", tag=f"km{c}") for c in range(2)]
            for c in range(2):
                k_ps = ps.tile([128, CH], F32, name="tl", tag="ps")
                for cc in range(2):
                    nc.tensor.matmul(k_ps[:], wk5[cc][:, c * 128:(c + 1) * 128],
                                     dst[:, 0, cc, :], start=(cc == 0), stop=(cc == 1))
                nc.scalar.activation(km[c][:], k_ps[:], AF.Identity, bias=b5[:, 2 + c:3 + c])
            o_ps = ps.tile([128, CH], F32, name="tl", tag="ps")
            for c in range(2):
                nc.vector.tensor_scalar_mul(km[c][:], km[c][:], qv[:, c:c + 1])
                nc.tensor.matmul(o_ps[0:64, :], ones64[:], km[c][:],
                                 start=(c == 0), stop=(c == 1))
            o_sb = sp_.tile([1, CH], F32, name="tl", tag="osb")
            nc.scalar.activation(o_sb[:], o_ps[0:1, :], AF.Copy, scale=1.0 / 32)
            nc.sync.dma_start(out_d[:], o_sb[:])

    nc.compile()
    return nc


def prep_inputs(inputs):
    inp = {k: np.ascontiguousarray(np.asarray(v)) for k, v in inputs.items()}
    pw, pb = inp['proj_w'].astype(np.float32), inp['proj_b'].astype(np.float32)
    mw, mb = inp['merge_w'].astype(np.float32), inp['merge_b'].astype(np.float32)
    w1, b1 = inp['mlp_w1'].astype(np.float32), inp['mlp_b1'].astype(np.float32)
    w2, b2 = inp['mlp_w2'].astype(np.float32), inp['mlp_b2'].astype(np.float32)
    ng, nb = inp['norm_g'].astype(np.float32), inp['norm_b'].astype(np.float32)

    wqkvT = np.empty((5, 3, 3, 256, 256), np.float32)
    mergeT = np.empty((5, 3, 256, 256), np.float32)
    w1T = np.empty((5, 3, 512, 512), np.float32)
    w2T = np.empty((5, 3, 512, 256), np.float32)
    pbq = np.empty((5, 3, 256), np.float32)
    pbk = np.empty((5, 3, 256), np.float32)
    pbv = np.empty((5, 3, 256), np.float32)
    for i in range(5):
        for br in range(3):
            for j in range(3):
                wqkvT[i, br, j] = pw[br, i, j][PERM].T
            wqkvT[i, br, 0] *= 0.125
            pbq[i, br] = pb[br, i, 0][PERM] * 0.125
            pbk[i, br] = pb[br, i, 1][PERM]
            pbv[i, br] = pb[br, i, 2][PERM]
            mergeT[i, br] = mw[br, i][:, PERM].T
            w1T[i, br] = w1[br, i].T
            w2T[i, br] = w2[br, i].T
    mbbv = np.transpose(mb[:, :5], (1, 0, 2)).astype(np.float32).copy()
    b1bv = np.transpose(b1[:, :5], (1, 0, 2)).astype(np.float32).copy()
    b2bv = np.transpose(b2[:, :5], (1, 0, 2)).astype(np.float32).copy()
    lngv = np.transpose(ng[:, :5], (1, 0, 2)).astype(np.float32).copy()
    lnbv = np.transpose(nb[:, :5], (1, 0, 2)).astype(np.float32).copy()
    w5T = np.stack([pw[0, 5, 0].T, pw[0, 5, 1].T]).astype(np.float32)
    pb5 = np.stack([pb[0, 5, 0], pb[0, 5, 1]]).astype(np.float32)

    desc = np.stack([inp[f'desc{t}'][0] for t in range(4)]).astype(np.float32)  # [4,256,N]
    bf = mybir.dt.np(mybir.dt.bfloat16)
    wqkvT = wqkvT.astype(bf); mergeT = mergeT.astype(bf)
    w1T = w1T.astype(bf); w2T = w2T.astype(bf)
    shared = dict(wqkvT=wqkvT, mergeT=mergeT, w1T=w1T, w2T=w2T, pbq=pbq, pbk=pbk,
                  pbv=pbv, mbb=mbbv, b1b=b1bv, b2b=b2bv, lng=lngv, lnb=lnbv,
                  w5T=w5T, pb5=pb5)
    in_maps = []
    for j in range(NC):
        xcj = desc[:, :, j * CH:(j + 1) * CH].reshape(4, 2, 128, CH)
        in_maps.append({"xc": np.ascontiguousarray(xcj), **shared})
    return in_maps


def kernel(**inputs):
    zb = all(not np.asarray(inputs[k]).any() for k in
             ("proj_b", "merge_b", "mlp_b1", "mlp_b2"))
    key = f"nc{zb}"
    if key not in _cache:
        _cache[key] = build_kernel(zb=zb)
    nc = _cache[key]
    in_maps = prep_inputs(inputs)
    res = run_bass_kernel_spmd(nc, in_maps, core_ids=list(range(NC)))
    out = np.concatenate([res.results[j]["out"][0] for j in range(NC)])
    mask = np.asarray(inputs["unreachable"]).any(axis=0)
    out = np.where(mask, np.float32(-1e9), out.astype(np.float32))
    return out
